# revision 1
# baseline (speedup 1.0000x reference)
"""Trainium2 Bass kernel for per-view cross-attention (v4).

Reference computation (per view v of 1024, S=64 samples, D=256):
  qp = q @ Wq.T + pe ; kp = k @ Wk.T + pe ; vp = v @ Wv.T + pe
  attn = softmax(qp @ kp.T / sqrt(D))
  x = gelu(attn @ vp @ Wo.T + bo) + q
Sharding: data-parallel over the 1024 views across 8 cores (128 views each).

Design notes:
- bf16 everywhere (DRAM I/O, SBUF, matmul operands; PSUM stays fp32).
  Halves HBM traffic and dodges the 4x cycles/row penalty on fp32 matmuls
  with small output free dims.
- Wo is folded into the v path on the host: vpo = v@(Wo@Wv).T + pe@Wo.T,
  so out = attn@vpo directly gives the pre-gelu activation (attn row-mixing
  commutes with Wo column-mixing). Kills the final projection entirely.
- scores are computed TRANSPOSED (operands swapped) and two views at a time
  with full 128-wide matmuls; only the two diagonal 64x64 blocks are valid,
  and the exp evacuation (one ACT op per half) reads just those.
- softmax reduction runs along partitions on PE: denominators via a [128,2]
  ones matmul (one per supertile), reciprocal on DVE, broadcast back across
  partitions with a [2,128] ones outer-product matmul (one per supertile).
- attn@vpo also runs two views per matmul using a block-diagonal normalized
  attnT tile ([128, 4, 128], off-diagonal zeroed once per tile by Pool).
- 4-stage software pipeline (A: load/proj/scoresT/exp; B: sums+recip;
  C: rec-broadcast+normalize; D: attn@vpo + gelu flush) so the in-order PE
  never waits on the ACT/DVE softmax chain. Stage A issues qp -> vpo -> kp
  so each PSUM ring slot has a full engine-burst of slack before reuse.
- engine balance per supertile (cost model): PE ~4.0us, DVE ~3.5, ACT ~3.6,
  Pool ~2.7. Pool (gpsimd) cannot access PSUM, so it gets SBUF-only work
  (residual adds, block-diag zeroing); pos-enc adds ride PSUM evacuations
  on DVE, except kp's, which is a third accumulation matmul on PE.
"""

import sys
import os

for p in ("/opt/trn_rl_repo",):
    if p not in sys.path and os.path.isdir(p):
        sys.path.insert(0, p)

import numpy as np
import ml_dtypes

BF16 = ml_dtypes.bfloat16

V, S, D = 1024, 64, 256
N_CORES = 8
VC = V // N_CORES          # views per core
ROWS = VC * S              # 8192 rows per core
R = 512                    # rows per supertile (8 views)
NST = ROWS // R            # supertiles per core
NV = R // S                # views per supertile
GELU_GROUP = 4             # supertiles per gelu flush (ACT table amortization)
LAG_B, LAG_C, LAG_D = 1, 2, 3
SCALE = 1.0 / np.sqrt(np.float32(D)).astype(np.float32)

_CACHE = {}


def _make_posenc(d_hid, n_samples):
    pos = np.arange(n_samples, dtype=np.float64)[:, None]
    j = np.arange(d_hid)[None, :]
    angle = pos / np.power(10000.0, 2.0 * (j // 2) / d_hid)
    table = np.where(j % 2 == 0, np.sin(angle), np.cos(angle))
    return table.astype(np.float32)  # [S, D]


def _build(rows=ROWS, gelu_copy=False, gelu_group=3, lags=(2, 3, 4),
           dep_hints=True, resid="split", qk_bufs=2, sm_bufs=3,
           psa_bufs=2, psb_bufs=3, store_q="sync", hoist_loads=True,
           ld_bufs=3):
    import concourse.bass as bass
    import concourse.mybir as mybir
    import concourse.tile as tile
    from concourse.tile import add_dep_helper
    from concourse import bacc
    from contextlib import ExitStack

    fp32 = mybir.dt.float32
    bf16 = mybir.dt.bfloat16
    AF = mybir.ActivationFunctionType
    ALU = mybir.AluOpType
    n_st = rows // R
    LAG_B, LAG_C, LAG_D = lags
    GG = gelu_group

    nc = bacc.Bacc(None, target_bir_lowering=False)

    qT_d = nc.dram_tensor("qT", [D, rows], bf16, kind="ExternalInput")
    kT_d = nc.dram_tensor("kT", [D, rows], bf16, kind="ExternalInput")
    vT_d = nc.dram_tensor("vT", [D, rows], bf16, kind="ExternalInput")
    wq_d = nc.dram_tensor("WqT", [D, D], bf16, kind="ExternalInput")
    wk_d = nc.dram_tensor("WkT", [D, D], bf16, kind="ExternalInput")
    wvo_d = nc.dram_tensor("WvoT", [D, D], bf16, kind="ExternalInput")
    bo_d = nc.dram_tensor("bo", [D], fp32, kind="ExternalInput")
    pet_d = nc.dram_tensor("peT_rep", [D, R], bf16, kind="ExternalInput")
    pen_d = nc.dram_tensor("pe_nat", [S, D], bf16, kind="ExternalInput")
    peo2_d = nc.dram_tensor("peo_nat2", [128, D], bf16, kind="ExternalInput")
    e8_d = nc.dram_tensor("E8", [S, R], bf16, kind="ExternalInput")
    ones2_d = nc.dram_tensor("ones2", [128, 2], bf16, kind="ExternalInput")
    ones2t_d = nc.dram_tensor("ones2T", [2, 128], bf16, kind="ExternalInput")
    out_d = nc.dram_tensor("outT", [D, rows], bf16, kind="ExternalOutput")

    def r3(ap):  # [D, X] dram -> [128, 2, X] partition view
        return ap.rearrange("(kc p) r -> p kc r", p=128)

    with tile.TileContext(nc) as tc, ExitStack() as ctx:
        ctx.enter_context(nc.allow_low_precision(
            reason="bf16 throughout is within the 2e-2 rel-err budget"))
        const = ctx.enter_context(tc.tile_pool(name="const", bufs=1))
        ld = ctx.enter_context(tc.tile_pool(name="ld", bufs=ld_bufs))
        proj = ctx.enter_context(tc.tile_pool(name="proj", bufs=2))
        sm = ctx.enter_context(tc.tile_pool(name="sm", bufs=sm_bufs))
        stg = ctx.enter_context(tc.tile_pool(name="stg", bufs=GG + 1))
        psA = ctx.enter_context(tc.tile_pool(name="psA", bufs=psa_bufs, space="PSUM"))
        psB = ctx.enter_context(tc.tile_pool(name="psB", bufs=psb_bufs, space="PSUM"))
        psS = ctx.enter_context(tc.tile_pool(name="psS", bufs=1, space="PSUM"))
        psR = ctx.enter_context(tc.tile_pool(name="psR", bufs=1, space="PSUM"))

        first = {}
        if hoist_loads:
            first["qt"] = ld.tile([128, 2, R], bf16, tag="qt",
                                  bufs=LAG_D + GG + 2, name="qt0")
            first["vt"] = ld.tile([128, 2, R], bf16, tag="vt", name="vt0")
            first["kt"] = ld.tile([128, 2, R], bf16, tag="kt", name="kt0")
            nc.sync.dma_start(first["qt"], r3(qT_d[:])[:, :, 0:R])
            nc.sync.dma_start(first["vt"], r3(vT_d[:])[:, :, 0:R])
            nc.sync.dma_start(first["kt"], r3(kT_d[:])[:, :, 0:R])
        wq = const.tile([128, 2, D], bf16)
        wk = const.tile([128, 2, D], bf16)
        wvo = const.tile([128, 2, D], bf16)
        nc.sync.dma_start(wq, r3(wq_d[:]))
        nc.sync.dma_start(wk, r3(wk_d[:]))
        nc.sync.dma_start(wvo, r3(wvo_d[:]))
        pet = const.tile([128, 2, R], bf16)
        nc.sync.dma_start(pet, r3(pet_d[:]))
        pen = const.tile([S, D], bf16)
        nc.sync.dma_start(pen, pen_d[:])
        peo2 = const.tile([128, D], bf16)
        nc.sync.dma_start(peo2, peo2_d[:])
        e8 = const.tile([S, R], bf16)
        nc.sync.dma_start(e8, e8_d[:])
        ones2 = const.tile([128, 2], bf16)
        nc.sync.dma_start(ones2, ones2_d[:])
        ones2t = const.tile([2, 128], bf16)
        nc.sync.dma_start(ones2t, ones2t_d[:])
        bo_sb = const.tile([128, 2], fp32)
        nc.sync.dma_start(bo_sb, bo_d.rearrange("(kc p) -> p kc", p=128))

        st_ctx = {}
        pending = []
        last_gelu = None
        last_exp = None
        for i in range(n_st + LAG_D):
            # -------- stage A: load, q/k/v projections, scoresT, exp --------
            if i < n_st:
                rs = slice(i * R, (i + 1) * R)
                if i == 0 and first:
                    qt, kt, vt = first["qt"], first["kt"], first["vt"]
                else:
                    qt = ld.tile([128, 2, R], bf16, tag="qt",
                                 bufs=LAG_D + GG + 2, name="qt")
                    kt = ld.tile([128, 2, R], bf16, tag="kt", name="kt")
                    vt = ld.tile([128, 2, R], bf16, tag="vt", name="vt")
                    nc.sync.dma_start(qt, r3(qT_d[:])[:, :, rs])
                    nc.sync.dma_start(vt, r3(vT_d[:])[:, :, rs])
                    nc.sync.dma_start(kt, r3(kT_d[:])[:, :, rs])

                # qp projection into transposed space qpT[dout, row];
                # pos-enc add rides the DVE evacuation.
                qpT = proj.tile([128, 2, R], bf16, tag="qpT", bufs=qk_bufs, name="qpT")
                kpT = proj.tile([128, 2, R], bf16, tag="kpT", bufs=qk_bufs, name="kpT")
                for mc in range(2):
                    ps = psA.tile([128, R], fp32, tag="psA", name="ps_qp")
                    for kc in range(2):
                        nc.tensor.matmul(
                            ps,
                            wq[:, kc, mc * 128:(mc + 1) * 128],
                            qt[:, kc, :],
                            start=(kc == 0),
                            stop=(kc == 1),
                        )
                    nc.vector.tensor_add(
                        out=qpT[:, mc, :], in0=ps, in1=pet[:, mc, :])

                # vpo = v@(Wo@Wv).T + pe@Wo.T, natural [row, dout] layout
                # (vt chunk as stationary); pos-enc add rides the evacuation.
                vpo = proj.tile([128, 4, D], bf16, tag="vpo",
                                bufs=LAG_D + 2, name="vpo")
                for g in range(4):
                    psv = psB.tile([128, 4, 128], fp32, tag="psB", name="ps_vpo")
                    pv = psv.rearrange("p a b -> p (a b)")[:, :D]
                    for kc in range(2):
                        nc.tensor.matmul(
                            pv,
                            vt[:, kc, g * 128:(g + 1) * 128],
                            wvo[:, kc, :],
                            start=(kc == 0),
                            stop=(kc == 1),
                        )
                    nc.vector.tensor_add(out=vpo[:, g, :], in0=pv, in1=peo2)

                # kp projection; pos-enc added on PE as a 3rd accumulation
                # matmul (pe_nat stationary, E8 one-hot rhs); ACT evacuates.
                for mc in range(2):
                    ps = psA.tile([128, R], fp32, tag="psA", name="ps_kp")
                    for kc in range(2):
                        nc.tensor.matmul(
                            ps,
                            wk[:, kc, mc * 128:(mc + 1) * 128],
                            kt[:, kc, :],
                            start=(kc == 0),
                            stop=False,
                        )
                    nc.tensor.matmul(
                        ps,
                        pen[:, mc * 128:(mc + 1) * 128],
                        e8,
                        start=False,
                        stop=True,
                    )
                    nc.scalar.copy(out=kpT[:, mc, :], in_=ps)

                # transposed scores, two views per matmul (full 128-wide):
                # scps[128(2 views k), g, 128(2 views q)]; only the diagonal
                # 64x64 blocks are meaningful.
                scps = psS.tile([128, 4, 128], fp32, tag="scps", name="scps")
                for g in range(4):
                    for dc in range(2):
                        nc.tensor.matmul(
                            scps[:, g, :],
                            kpT[:, dc, g * 128:(g + 1) * 128],
                            qpT[:, dc, g * 128:(g + 1) * 128],
                            start=(dc == 0),
                            stop=(dc == 1),
                        )

                # exp of the diagonal blocks only, into compact attnu
                # (no max-subtraction: |scores/16| < ~10)
                attnu = sm.tile([128, 4, S], bf16, tag="attnu",
                                bufs=LAG_C + 2, name="attnu")
                for h in range(2):
                    hs = slice(h * 64, (h + 1) * 64)
                    _e = nc.scalar.activation(
                        attnu[hs, :, :], scps[hs, :, h * 64:(h + 1) * 64],
                        AF.Exp, scale=float(SCALE))
                    if dep_hints and last_gelu is not None:
                        add_dep_helper(_e.ins, last_gelu, sync=False,
                                       reason="act-table grouping: exp after prior gelus")
                    last_exp = _e.ins
                st_ctx[i] = dict(qt=qt, vpo=vpo, attnu=attnu)

            # -------- stage B: softmax denominators (PE) + reciprocal ------
            jb = i - LAG_B
            if 0 <= jb < n_st:
                c = st_ctx[jb]
                sums = psR.tile([2, 4, S], fp32, tag="sums", name="sums",
                                padded_shape=[2, 4, 2 * S])
                nc.tensor.matmul(sums, ones2, c["attnu"], start=True, stop=True)
                rec = sm.tile([2, 4, S], bf16, tag="rec", name="rec")
                nc.vector.reciprocal(rec, sums)
                c["rec"] = rec

            # -------- stage C: broadcast reciprocal, normalize into
            # block-diagonal attnT2 (off-diagonal zeroed by Pool) -----------
            jc = i - LAG_C
            if 0 <= jc < n_st:
                c = st_ctx[jc]
                rrep = psR.tile([128, 4, S], fp32, tag="rrep", name="rrep",
                                padded_shape=[128, 4, 2 * S])
                nc.tensor.matmul(rrep, ones2t, c["rec"], start=True, stop=True)
                attnT2 = sm.tile([128, 4, 128], bf16, tag="attnT2", name="attnT2")
                nc.gpsimd.memset(attnT2, 0.0)
                for h in range(2):
                    hs = slice(h * 64, (h + 1) * 64)
                    nc.vector.tensor_tensor(
                        attnT2[hs, :, h * 64:(h + 1) * 64],
                        c["attnu"][hs, :, :], rrep[hs, :, :], ALU.mult)
                c["attnT2"] = attnT2

            # -------- stage D: attn@vpo -> pre-gelu, gelu flush ------------
            jd = i - LAG_D
            if 0 <= jd < n_st:
                c = st_ctx.pop(jd)
                pre = stg.tile([128, 2, R], bf16, tag="pre", name="pre")
                for cc in range(2):
                    pso = psB.tile([128, 4, 128], fp32, tag="psB", name="ps_av")
                    for g in range(4):
                        nc.tensor.matmul(
                            pso[:, g, :],
                            c["vpo"][:, g, cc * 128:(cc + 1) * 128],
                            c["attnT2"][:, g, :],
                            start=True,
                            stop=True,
                        )
                    # pso free layout [g, (two h, s)] == pre chunk layout
                    if cc == 0:
                        nc.scalar.copy(out=pre[:, cc, :],
                                       in_=pso.rearrange("p a b -> p (a b)"))
                    else:
                        nc.vector.tensor_copy(
                            pre[:, cc, :], pso.rearrange("p a b -> p (a b)"))
                pending.append((jd, pre, c["qt"]))

                if len(pending) == GG or jd == n_st - 1:
                    outs = []
                    for pst, ppre, pqt in pending:
                        outsb = proj.tile([128, 2, R], bf16, tag="outsb",
                                          bufs=GG + 1, name="outsb")
                        for mc in range(2):
                            if gelu_copy:
                                _g = nc.scalar.activation(
                                    out=outsb[:, mc, :], in_=ppre[:, mc, :],
                                    func=AF.Copy, bias=0.0, scale=1.0,
                                )
                            else:
                                _g = nc.scalar.activation(
                                    out=outsb[:, mc, :], in_=ppre[:, mc, :],
                                    func=AF.Gelu, bias=bo_sb[:, mc:mc + 1],
                                    scale=1.0,
                                )
                            if dep_hints and last_exp is not None:
                                add_dep_helper(_g.ins, last_exp, sync=False,
                                               reason="act-table grouping: gelu after group exps")
                            last_gelu = _g.ins
                            r_eng = (nc.gpsimd if resid == "pool" else
                                     nc.vector if resid == "dve" else
                                     (nc.vector if mc == 0 else nc.gpsimd))
                            r_eng.tensor_add(
                                out=outsb[:, mc, :], in0=outsb[:, mc, :],
                                in1=pqt[:, mc, :],
                            )
                        outs.append((pst, outsb))
                    st_eng = nc.scalar if store_q == "act" else nc.sync
                    for pst, outsb in outs:
                        st_eng.dma_start(
                            r3(out_d[:])[:, :, pst * R:(pst + 1) * R], outsb
                        )
                    pending = []

    nc.finalize()
    return nc


def _get_nc():
    if "nc" not in _CACHE:
        _CACHE["nc"] = _build()
    return _CACHE["nc"]


def _host_inputs(q, k, v, Wq, Wk, Wv, Wo, bo):
    pe = _make_posenc(D, S)                                   # [S, D] fp32
    Wo32 = np.asarray(Wo, np.float32)
    Wv32 = np.asarray(Wv, np.float32)
    Wvo = Wo32 @ Wv32                                         # fused v->out
    peo = pe @ Wo32.T                                         # pe through Wo
    peT_rep = np.ascontiguousarray(np.tile(pe.T, (1, NV))).astype(BF16)
    peo2 = np.ascontiguousarray(np.tile(peo, (2, 1))).astype(BF16)  # [128, D]
    e8 = np.ascontiguousarray(
        np.tile(np.eye(S, dtype=np.float32), (1, NV))).astype(BF16)  # [S, R]
    ones2 = np.zeros((128, 2), BF16)
    ones2[:64, 0] = 1
    ones2[64:, 1] = 1
    ones2t = np.ascontiguousarray(ones2.T)                    # [2, 128]
    consts = {
        "WqT": np.asarray(Wq, np.float32).T.astype(BF16),
        "WkT": np.asarray(Wk, np.float32).T.astype(BF16),
        "WvoT": Wvo.T.astype(BF16),
        "bo": np.ascontiguousarray(np.asarray(bo, np.float32)),
        "peT_rep": peT_rep,
        "pe_nat": pe.astype(BF16),
        "peo_nat2": peo2,
        "E8": e8,
        "ones2": ones2,
        "ones2T": ones2t,
    }
    consts = {k_: np.ascontiguousarray(v_) for k_, v_ in consts.items()}
    qb = np.asarray(q, np.float32).astype(BF16)
    kb = np.asarray(k, np.float32).astype(BF16)
    vb = np.asarray(v, np.float32).astype(BF16)
    in_maps = []
    for c in range(N_CORES):
        sl = slice(c * VC, (c + 1) * VC)
        m = dict(consts)
        m["qT"] = np.ascontiguousarray(qb[sl].reshape(ROWS, D).T)
        m["kT"] = np.ascontiguousarray(kb[sl].reshape(ROWS, D).T)
        m["vT"] = np.ascontiguousarray(vb[sl].reshape(ROWS, D).T)
        in_maps.append(m)
    return in_maps


def kernel(q, k, v, Wq, Wk, Wv, Wo, bo, _trace=False):
    from concourse.bass_utils import run_bass_kernel_spmd

    nc = _get_nc()
    in_maps = _host_inputs(q, k, v, Wq, Wk, Wv, Wo, bo)
    res = run_bass_kernel_spmd(nc, in_maps, list(range(N_CORES)), trace=_trace)
    outs = [
        np.asarray(res.results[c]["outT"], np.float32)
        .reshape(D, VC, S).transpose(1, 2, 0)
        for c in range(N_CORES)
    ]
    full = np.concatenate(outs, axis=0)
    if _trace:
        _CACHE["last_results"] = res
    return full



# revision 9
# speedup vs baseline: 1.0018x; 1.0018x over previous
"""Trainium2 Bass kernel for per-view cross-attention (v5a).

Reference computation (per view v of 1024, S=64 samples, D=256):
  qp = q @ Wq.T + pe ; kp = k @ Wk.T + pe ; vp = v @ Wv.T + pe
  attn = softmax(qp @ kp.T / sqrt(D))
  x = gelu(attn @ vp @ Wo.T + bo) + q
Sharding: data-parallel over the 1024 views across 8 cores (128 views each).

v5a design notes (vs v4 baseline at 102.0us):
- q-projection folded into the k side: scoresT[k, q] =
    kpw_k . q_q  +  kCk[k, s_q]  +  PP[s_k, s_q]
  with kpw = k@(Wk.T@Wq) + pe@Wq (pe-add folded into the evacuations),
  Ck = Wk.T@pe.T, PP = pe@pe.T (symmetric; rides the E8 one-hot matmul
  with kCkT). Raw qT feeds the scores matmul directly (it is resident
  anyway for the residual), so qp is never materialized.
- unnormalized softmax: exp writes block-diagonal attnu into a Pool-
  zeroed tile; denominators via a ones-column matmul (free size 512);
  reciprocal broadcast back across partitions with a 1-row ones matmul
  (rec2, PSUM); attnu normalized IN-PLACE on DVE (SBUF x PSUM - the DVE
  has a single PSUM read port, so pso*rec2 at stage D is illegal).
- bo folded into peo (softmax rows sum to 1, and with unnormalized
  attnu: attnu@(vpo+bo)*rec = attnu@vpo*rec + bo exactly), so gelu has
  no per-chunk bias and runs as ONE merged 1024-free op per supertile.
- DMA count cut ~3x: q/k/v loaded in 4-supertile chunks, output stored
  once per gelu flush group (the SP sequencer + single HWDGE device
  serialize at ~650ns per DMA; v4's 74 DMAs were a hidden span floor).
- engine assignment (cost model, per 512-row supertile): PE 3.84us
  (projections incl. half the peo adds via E8, scores, sums, rec2,
  attn@v), DVE 3.95 (kpw-mc0/kCk/vpo-g01 evacuations with const folds,
  reciprocal, in-place normalize, stage-D cc0 copy), ACT 3.99 (exp,
  merged gelu, kpw-mc1/vpo-g23/stage-D cc1 copies, 4 table loads),
  Pool 3.29 (Pq-mc1 add, residuals, attnu memset).
"""

import sys
import os

for p in ("/opt/trn_rl_repo",):
    if p not in sys.path and os.path.isdir(p):
        sys.path.insert(0, p)

import numpy as np
import ml_dtypes

BF16 = ml_dtypes.bfloat16

V, S, D = 1024, 64, 256
N_CORES = 8
VC = V // N_CORES          # views per core
ROWS = VC * S              # 8192 rows per core
R = 512                    # rows per supertile (8 views)
NST = ROWS // R            # supertiles per core
NV = R // S                # views per supertile
SCALE = 1.0 / np.sqrt(np.float32(D)).astype(np.float32)

_CACHE = {}


def _make_posenc(d_hid, n_samples):
    pos = np.arange(n_samples, dtype=np.float64)[:, None]
    j = np.arange(d_hid)[None, :]
    angle = pos / np.power(10000.0, 2.0 * (j // 2) / d_hid)
    table = np.where(j % 2 == 0, np.sin(angle), np.cos(angle))
    return table.astype(np.float32)  # [S, D]


def _build(rows=ROWS, lags=(1, 2, 2), groups=(7, 6, 3), load_chunk=2,
           kpw_mc1_act=False, vpo_act_half=True, peo_pe_half=True,
           sd_act=2, resid="alt", memset_eng="dve"):
    import concourse.bass as bass
    import concourse.mybir as mybir
    import concourse.tile as tile
    from concourse.tile import add_dep_helper
    from concourse import bacc
    from contextlib import ExitStack

    fp32 = mybir.dt.float32
    bf16 = mybir.dt.bfloat16
    AF = mybir.ActivationFunctionType
    ALU = mybir.AluOpType
    n_st = rows // R
    LAG_B, LAG_C, LAG_D = lags
    GGMAX = max(groups)
    LC = load_chunk
    assert sum(groups) == n_st
    assert n_st % LC == 0
    flush_at = {}
    acc = 0
    for g in groups:
        start = acc
        acc += g
        flush_at[acc - 1] = (start, g)

    nc = bacc.Bacc(None, target_bir_lowering=False)

    qT_d = nc.dram_tensor("qT", [D, rows], bf16, kind="ExternalInput")
    kT_d = nc.dram_tensor("kT", [D, rows], bf16, kind="ExternalInput")
    vT_d = nc.dram_tensor("vT", [D, rows], bf16, kind="ExternalInput")
    b_d = nc.dram_tensor("B", [D, D], bf16, kind="ExternalInput")
    ck_d = nc.dram_tensor("Ck", [D, S], bf16, kind="ExternalInput")
    wvo_d = nc.dram_tensor("WvoT", [D, D], bf16, kind="ExternalInput")
    pqt_d = nc.dram_tensor("PqT_rep", [D, R], bf16, kind="ExternalInput")
    pp_d = nc.dram_tensor("PP_rep", [S, R], bf16, kind="ExternalInput")
    peo2_d = nc.dram_tensor("peo2", [128, 2 * D], bf16, kind="ExternalInput")
    peon_d = nc.dram_tensor("peo_nat", [S, D], bf16, kind="ExternalInput")
    e8_d = nc.dram_tensor("E8", [S, R], bf16, kind="ExternalInput")
    ones1_d = nc.dram_tensor("ones1", [128, 1], bf16, kind="ExternalInput")
    ones1t_d = nc.dram_tensor("ones1T", [1, 128], bf16, kind="ExternalInput")
    out_d = nc.dram_tensor("outT", [D, rows], bf16, kind="ExternalOutput")

    def r3(ap):  # [D, X] dram -> [128, 2, X] partition view
        return ap.rearrange("(kc p) r -> p kc r", p=128)

    # enough load-chunk buffers that qt survives until its flush group ends
    qt_bufs = (LAG_D + GGMAX) // LC + 2
    kv_bufs = 2

    with tile.TileContext(nc) as tc, ExitStack() as ctx:
        ctx.enter_context(nc.allow_low_precision(
            reason="bf16 throughout is within the 2e-2 rel-err budget"))
        const = ctx.enter_context(tc.tile_pool(name="const", bufs=1))
        ld = ctx.enter_context(tc.tile_pool(name="ld", bufs=2))
        proj = ctx.enter_context(tc.tile_pool(name="proj", bufs=2))
        sm = ctx.enter_context(tc.tile_pool(name="sm", bufs=3))
        stg = ctx.enter_context(tc.tile_pool(name="stg", bufs=GGMAX + 1))
        psA = ctx.enter_context(tc.tile_pool(name="psA", bufs=2, space="PSUM"))
        psVO = ctx.enter_context(tc.tile_pool(name="psVO", bufs=3, space="PSUM"))
        psS = ctx.enter_context(tc.tile_pool(name="psS", bufs=1, space="PSUM"))
        psR = ctx.enter_context(tc.tile_pool(name="psR", bufs=2, space="PSUM"))

        bsb = const.tile([128, 2, D], bf16)
        cksb = const.tile([128, 2, S], bf16)
        wvo = const.tile([128, 2, D], bf16)
        nc.scalar.dma_start(bsb, r3(b_d[:]))
        nc.scalar.dma_start(cksb, r3(ck_d[:]))
        nc.scalar.dma_start(wvo, r3(wvo_d[:]))
        pqt = const.tile([128, 2, R], bf16)
        nc.scalar.dma_start(pqt, r3(pqt_d[:]))
        pp = const.tile([S, R], bf16)
        nc.scalar.dma_start(pp, pp_d[:])
        peo2 = const.tile([128, 2 * D], bf16)
        nc.scalar.dma_start(peo2, peo2_d[:])
        peon = const.tile([S, D], bf16)
        nc.scalar.dma_start(peon, peon_d[:])
        e8 = const.tile([S, R], bf16)
        nc.scalar.dma_start(e8, e8_d[:])
        ones1 = const.tile([128, 1], bf16)
        nc.scalar.dma_start(ones1, ones1_d[:])
        ones1t = const.tile([1, 128], bf16)
        nc.scalar.dma_start(ones1t, ones1t_d[:])

        # persistent block-diagonal attnu ring: exp rewrites only the
        # diagonal blocks and the in-place normalize keeps zeros zero, so
        # each slot is zeroed exactly once.
        attnu_ring = []
        for ri in range(LAG_D + 2):
            t = const.tile([128, 4, 128], bf16, name=f"attnu{ri}")
            nc.gpsimd.memset(t.rearrange("p a b -> p (a b)"), 0.0)
            attnu_ring.append(t)

        chunks = {}

        def load_chunk_for(i):
            ch = i // LC
            if ch in chunks:
                return chunks[ch]
            cs = slice(ch * LC * R, (ch + 1) * LC * R)
            qtc = ld.tile([128, 2, LC * R], bf16, tag="qt", bufs=qt_bufs,
                          name="qtc")
            ktc = ld.tile([128, 2, LC * R], bf16, tag="kt", bufs=kv_bufs,
                          name="ktc")
            vtc = ld.tile([128, 2, LC * R], bf16, tag="vt", bufs=kv_bufs,
                          name="vtc")
            nc.sync.dma_start(ktc, r3(kT_d[:])[:, :, cs])
            nc.sync.dma_start(qtc, r3(qT_d[:])[:, :, cs])
            nc.sync.dma_start(vtc, r3(vT_d[:])[:, :, cs])
            chunks[ch] = (qtc, ktc, vtc)
            return chunks[ch]

        st_ctx = {}
        pending = []
        gout = None
        last_gelu = None
        last_exp = None
        for i in range(n_st + LAG_D):
            # ---- stage A: load, kpw/kCk/vpo projections, scoresT, exp ----
            if i < n_st:
                qtc, ktc, vtc = load_chunk_for(i)
                if i % LC == 0 and i + LC < n_st:
                    load_chunk_for(i + LC)  # prefetch next chunk
                off = (i % LC) * R
                ss = slice(off, off + R)
                qt = qtc[:, :, ss]
                kt = ktc[:, :, ss]
                vt = vtc[:, :, ss]

                # kpw = k@B + Pq (pe@Wq), transposed layout [dout, row].
                # mc0: DVE evacuation with the Pq-add folded in.
                # mc1: ACT pure copy, Pq-add on Pool (SBUF-only).
                kpwT = proj.tile([128, 2, R], bf16, tag="kpwT", name="kpwT")
                for mc in range(2):
                    ps = psA.tile([128, R], fp32, tag="psA", name="ps_kpw")
                    for kc in range(2):
                        nc.tensor.matmul(
                            ps,
                            bsb[:, kc, mc * 128:(mc + 1) * 128],
                            kt[:, kc, :],
                            start=(kc == 0),
                            stop=(kc == 1),
                        )
                    if kpw_mc1_act and mc == 1:
                        nc.scalar.copy(out=kpwT[:, mc, :], in_=ps)
                        nc.gpsimd.tensor_add(
                            out=kpwT[:, mc, :], in0=kpwT[:, mc, :],
                            in1=pqt[:, mc, :])
                    else:
                        nc.vector.tensor_add(
                            out=kpwT[:, mc, :], in0=ps, in1=pqt[:, mc, :])

                # kCkT = (k@Ck).T [s, row]; PP-fold (pe@pe.T, symmetric)
                # rides the DVE evacuation. PSUM tile shares the psA ring.
                kckT = proj.tile([S, R], bf16, tag="kckT", name="kckT")
                psk = psA.tile([128, R], fp32, tag="psA", name="ps_kck")
                for kc in range(2):
                    nc.tensor.matmul(
                        psk[0:S, :],
                        cksb[:, kc, :],
                        kt[:, kc, :],
                        start=(kc == 0),
                        stop=(kc == 1),
                    )
                nc.vector.tensor_add(out=kckT, in0=psk[0:S, :], in1=pp)

                # vpo = v@Wvo.T + peo' (peo' = pe@Wo.T + bo), natural
                # [row, dout] layout, two g-blocks per PSUM tile.
                vpo = proj.tile([128, 4, D], bf16, tag="vpo",
                                bufs=LAG_D + 2, name="vpo")
                for gp in range(2):
                    psv = psVO.tile([128, 2, D], fp32, tag="psVO", name="ps_vpo")
                    for gi in range(2):
                        g = gp * 2 + gi
                        for kc in range(2):
                            nc.tensor.matmul(
                                psv[:, gi, :],
                                vt[:, kc, g * 128:(g + 1) * 128],
                                wvo[:, kc, :],
                                start=(kc == 0),
                                stop=(kc == 1) if not (peo_pe_half and gp == 1)
                                else False,
                            )
                        if peo_pe_half and gp == 1:
                            # peo-add as a 3rd accumulation matmul (E8 rhs)
                            nc.tensor.matmul(
                                psv[:, gi, :],
                                e8[:, g * 128:(g + 1) * 128],
                                peon,
                                start=False,
                                stop=True,
                            )
                    if vpo_act_half and gp == 1:
                        nc.scalar.copy(
                            out=vpo[:, gp * 2:(gp + 1) * 2, :],
                            in_=psv,
                        )
                    else:
                        nc.vector.tensor_add(
                            out=vpo[:, gp * 2:(gp + 1) * 2, :],
                            in0=psv,
                            in1=peo2.rearrange("p (a d) -> p a d", a=2),
                        )

                # scoresT[k, q] per g-block: kpw.q (2 d-chunks) + one-hot
                # E8 matmul carrying kCk + PP.
                scps = psS.tile([128, 4, 128], fp32, tag="scps", name="scps")
                for g in range(4):
                    gs = slice(g * 128, (g + 1) * 128)
                    for dc in range(2):
                        nc.tensor.matmul(
                            scps[:, g, :],
                            kpwT[:, dc, gs],
                            qt[:, dc, gs],
                            start=(dc == 0),
                            stop=False,
                        )
                    nc.tensor.matmul(
                        scps[:, g, :],
                        kckT[:, gs],
                        e8[:, gs],
                        start=False,
                        stop=True,
                    )

                # block-diagonal unnormalized attn: Pool zeroes the tile,
                # exp fills the two diagonal 64x64 block-columns.
                attnu = attnu_ring[i % (LAG_D + 2)]
                for h in range(2):
                    hs = slice(h * 64, (h + 1) * 64)
                    _e = nc.scalar.activation(
                        attnu[hs, :, hs], scps[hs, :, hs],
                        AF.Exp, scale=float(SCALE))
                    if last_gelu is not None:
                        add_dep_helper(_e.ins, last_gelu, sync=False,
                                       reason="act-table grouping")
                    last_exp = _e.ins
                st_ctx[i] = dict(qt=qt, vpo=vpo, attnu=attnu)

            # ---- stage B: denominators (ones-column matmul) + recip ----
            jb = i - LAG_B
            if 0 <= jb < n_st:
                c = st_ctx[jb]
                sr = psR.tile([128, 4, 128], fp32, tag="sr", name="sr")
                nc.tensor.matmul(sr[0:1, :, :], ones1, c["attnu"],
                                 start=True, stop=True)
                rec1 = sm.tile([1, 4, 128], bf16, tag="rec1", name="rec1")
                nc.vector.reciprocal(rec1, sr[0:1, :, :])
                c["rec1"] = rec1
                c["sr"] = sr

            # ---- stage C: broadcast reciprocal, normalize attnu in-place --
            jc = i - LAG_C
            if 0 <= jc < n_st:
                c = st_ctx[jc]
                nc.tensor.matmul(c["sr"], ones1t, c["rec1"],
                                 start=True, stop=True)
                af = c["attnu"].rearrange("p a b -> p (a b)")
                nc.vector.tensor_tensor(
                    af, af, c["sr"].rearrange("p a b -> p (a b)"), ALU.mult)

            # ---- stage D: attn@vpo, evacuate, gelu flush ----
            jd = i - LAG_D
            if 0 <= jd < n_st:
                c = st_ctx.pop(jd)
                pre = stg.tile([128, 2, R], bf16, tag="pre",
                               bufs=GGMAX + 1, name="pre")
                for cc in range(2):
                    pso = psVO.tile([128, 4, 128], fp32, tag="psVO",
                                    name="ps_av")
                    for g in range(4):
                        nc.tensor.matmul(
                            pso[:, g, :],
                            c["vpo"][:, g, cc * 128:(cc + 1) * 128],
                            c["attnu"][:, g, :],
                            start=True,
                            stop=True,
                        )
                    if cc >= 2 - sd_act:
                        nc.scalar.copy(
                            out=pre[:, cc, :],
                            in_=pso.rearrange("p a b -> p (a b)"))
                    else:
                        nc.vector.tensor_copy(
                            pre[:, cc, :], pso.rearrange("p a b -> p (a b)"))
                pending.append((jd, pre, c["qt"]))

                if jd in flush_at:
                    g0, gn = flush_at[jd]
                    gout = stg.tile([128, 2, GGMAX * R], bf16, tag="gout",
                                    bufs=1, name="gout")
                    for idx, (pst, ppre, pqt_t) in enumerate(pending):
                        osl = slice(idx * R, (idx + 1) * R)
                        _g = nc.scalar.activation(
                            out=gout[:, :, osl],
                            in_=ppre,
                            func=AF.Gelu, bias=0.0, scale=1.0,
                        )
                        if last_exp is not None:
                            add_dep_helper(_g.ins, last_exp, sync=False,
                                           reason="act-table grouping")
                        last_gelu = _g.ins
                        if resid == "alt":
                            r_eng = nc.gpsimd if idx % 2 == 0 else nc.vector
                        else:
                            r_eng = nc.gpsimd if resid == "pool" else nc.vector
                        r_eng.tensor_add(
                            out=gout[:, :, osl],
                            in0=gout[:, :, osl],
                            in1=pqt_t,
                        )
                    if jd == n_st - 1:
                        for idx in range(gn):
                            nc.sync.dma_start(
                                r3(out_d[:])[:, :, (g0 + idx) * R:
                                             (g0 + idx + 1) * R],
                                gout[:, :, idx * R:(idx + 1) * R],
                            )
                    else:
                        nc.sync.dma_start(
                            r3(out_d[:])[:, :, g0 * R:(g0 + gn) * R],
                            gout[:, :, 0:gn * R],
                        )
                    pending = []

    nc.finalize()
    return nc


def _get_nc():
    if "nc" not in _CACHE:
        _CACHE["nc"] = _build()
    return _CACHE["nc"]


def _host_inputs(q, k, v, Wq, Wk, Wv, Wo, bo):
    pe = _make_posenc(D, S)                                   # [S, D] fp32
    Wq32 = np.asarray(Wq, np.float32)
    Wk32 = np.asarray(Wk, np.float32)
    Wv32 = np.asarray(Wv, np.float32)
    Wo32 = np.asarray(Wo, np.float32)
    bo32 = np.asarray(bo, np.float32)

    B = Wk32.T @ Wq32                                         # [D, D]
    Pq = pe @ Wq32                                            # [S, D]
    Ck = Wk32.T @ pe.T                                        # [D, S]
    PP = pe @ pe.T                                            # [S, S]
    Wvo = Wo32 @ Wv32                                         # [D, D]
    peo = pe @ Wo32.T + bo32[None, :]                         # [S, D] +bo fold

    pqt_rep = np.tile(Pq.T, (1, NV))                          # [D, R]
    pp_rep = np.tile(PP, (1, NV))                             # [S, R]
    peo2 = np.tile(np.tile(peo, (2, 1)), (1, 2))              # [128, 2D]
    e8 = np.tile(np.eye(S, dtype=np.float32), (1, NV))        # [S, R]
    consts = {
        "B": B.astype(BF16),
        "Ck": Ck.astype(BF16),
        "WvoT": Wvo.T.astype(BF16),
        "PqT_rep": pqt_rep.astype(BF16),
        "PP_rep": pp_rep.astype(BF16),
        "peo2": peo2.astype(BF16),
        "peo_nat": peo.astype(BF16),
        "E8": e8.astype(BF16),
        "ones1": np.ones((128, 1), np.float32).astype(BF16),
        "ones1T": np.ones((1, 128), np.float32).astype(BF16),
    }
    consts = {k_: np.ascontiguousarray(v_) for k_, v_ in consts.items()}
    qb = np.asarray(q, np.float32).astype(BF16)
    kb = np.asarray(k, np.float32).astype(BF16)
    vb = np.asarray(v, np.float32).astype(BF16)
    in_maps = []
    for c in range(N_CORES):
        sl = slice(c * VC, (c + 1) * VC)
        m = dict(consts)
        m["qT"] = np.ascontiguousarray(qb[sl].reshape(ROWS, D).T)
        m["kT"] = np.ascontiguousarray(kb[sl].reshape(ROWS, D).T)
        m["vT"] = np.ascontiguousarray(vb[sl].reshape(ROWS, D).T)
        in_maps.append(m)
    return in_maps


def kernel(q, k, v, Wq, Wk, Wv, Wo, bo, _trace=False):
    from concourse.bass_utils import run_bass_kernel_spmd

    nc = _get_nc()
    in_maps = _host_inputs(q, k, v, Wq, Wk, Wv, Wo, bo)
    res = run_bass_kernel_spmd(nc, in_maps, list(range(N_CORES)), trace=_trace)
    outs = [
        np.asarray(res.results[c]["outT"], np.float32)
        .reshape(D, VC, S).transpose(1, 2, 0)
        for c in range(N_CORES)
    ]
    full = np.concatenate(outs, axis=0)
    if _trace:
        _CACHE["last_results"] = res
    return full
